# revision 1
# baseline (speedup 1.0000x reference)
"""ResNet BasicBlock forward on 8 Trainium2 NeuronCores.

Computes relu(bn2(conv2(relu(bn1(conv1(x))))) + x) for x[64,128,56,56],
two 3x3 stride-1 pad-1 convs with 128->128 channels, eval-mode BN.

Strategy:
  - Data parallel over batch: 8 images per core, no collectives.
  - Each 3x3 conv = 9 shifted matmuls accumulated in PSUM. Input channels
    (128) sit on the SBUF partition dim (= matmul contraction dim); output
    channels land on PSUM partitions. Spatial output is tiled into 7 PSUM
    banks of 8 rows x 56 cols (448 fp32 = one 2KB bank).
  - bf16 matmul inputs (1 cycle/row on the PE vs 4 for fp32), fp32 PSUM
    accumulation. x is cast to bf16 on the host and DMAd straight into a
    58x58 zero-bordered SBUF buffer, so every shifted 3x3 tap is a plain
    strided access pattern and padding costs nothing. BN scale is folded
    into the conv weights on the host; BN shift is a per-channel bias.
  - Epilogues: ScalarE does relu(psum + b1) -> bf16 mid (padded); VectorE
    does (psum + b2) + residual then relu. The residual is read from the
    bf16 input buffer (adds <0.1% error, saves a 12.8MB/core fp32 stream).
  - Ramp/tail: image 0's input DMA is split into 7 row chunks so the first
    matmul starts as soon as ~2 chunks land; a dummy activation hoists the
    ACT table load off the critical path; the last image's output DMA is
    chunked per bank. The Tile framework inserts all semaphores; images
    are software-pipelined DEPTH deep.
"""

import functools
import os
import sys

import numpy as np

for _p in ("/opt/trn_rl_repo", "/root/.axon_site/_ro/trn_rl_repo"):
    if os.path.isdir(_p) and _p not in sys.path:
        sys.path.append(_p)

import ml_dtypes  # noqa: E402

import concourse.bass as bass  # noqa: E402,F401
import concourse.mybir as mybir  # noqa: E402
import concourse.tile as tile  # noqa: E402
from concourse import bacc, bass_utils  # noqa: E402

N_CORES = 8
IMGS = 8  # images per core
C = 128
H = W = 56
HP = WP = 58  # padded spatial
RPB = 8  # output rows per PSUM bank
BANKS = H // RPB  # 7
KK = 9  # 3x3 taps
EPS = 1e-5
DEPTH = 4  # image pipeline depth

BF16 = mybir.dt.bfloat16
F32 = mybir.dt.float32


def _build_module():
    nc = bacc.Bacc(
        "TRN2",
        target_bir_lowering=False,
        debug=False,
        enable_asserts=False,
        num_devices=N_CORES,
        enable_partition_id=False,
    )
    xbf_d = nc.dram_tensor("xbf", [IMGS, C, H, W], BF16, kind="ExternalInput").ap()
    w1_d = nc.dram_tensor("w1t", [C, KK, C], BF16, kind="ExternalInput").ap()
    w2_d = nc.dram_tensor("w2t", [C, KK, C], BF16, kind="ExternalInput").ap()
    b1_d = nc.dram_tensor("b1", [C, 1], F32, kind="ExternalInput").ap()
    b2_d = nc.dram_tensor("b2", [C, 1], F32, kind="ExternalInput").ap()
    out_d = nc.dram_tensor("out", [IMGS, C, H, W], F32, kind="ExternalOutput").ap()

    add = mybir.AluOpType.add
    relu = mybir.ActivationFunctionType.Relu

    with tile.TileContext(nc) as tc:
        with (
            tc.tile_pool(name="singles", bufs=1) as singles,
            tc.tile_pool(name="psum", bufs=8, space="PSUM") as psum_pool,
        ):
            w1_sb = singles.tile([C, KK, C], BF16, name="w1_sb")
            w2_sb = singles.tile([C, KK, C], BF16, name="w2_sb")
            b1_sb = singles.tile([C, 1], F32, name="b1_sb")
            b2_sb = singles.tile([C, 1], F32, name="b2_sb")
            dummy = singles.tile([C, 1], F32, name="dummy")
            warm = singles.tile([C, 448], BF16, name="warm")

            nc.vector.memset(dummy, 0.0)
            nc.vector.memset(warm, 0.0)

            x_pad = [
                singles.tile([C, HP, WP], BF16, name=f"x_pad{d}") for d in range(DEPTH)
            ]
            mid_pad = [
                singles.tile([C, HP, WP], BF16, name=f"mid_pad{d}")
                for d in range(DEPTH)
            ]
            out_sb = [
                singles.tile([C, H, W], F32, name=f"out_sb{d}") for d in range(DEPTH)
            ]

            def zero_borders(buf):
                nc.gpsimd.memset(buf[:, 0, :], 0.0)
                nc.gpsimd.memset(buf[:, HP - 1, :], 0.0)
                nc.gpsimd.memset(buf[:, 1 : HP - 1, 0 : WP : WP - 1], 0.0)

            # Borders image 0 needs, ahead of the GpSimd scatter copies.
            zero_borders(x_pad[0])
            zero_borders(mid_pad[0])

            # Image 0's input goes through a contiguous staging tile (the
            # padded-interior scatter DMA moves 112-byte rows and is ~4x
            # slower per byte): w1 first on the sync HWDGE queue, then 4
            # contiguous row-chunks of image 0. GpSimd (otherwise idle)
            # scatters them into the padded buffer in 8-row chunks.
            # Image 0's input goes through a contiguous staging tile (the
            # padded-interior scatter DMA moves 112-byte rows and is much
            # slower per byte): w1 first on the sync HWDGE queue, then 4
            # contiguous row-chunks of image 0, scattered into the padded
            # buffer by VectorE (idle during the ramp).
            xq = singles.tile([C, H, W], BF16, name="xq")
            nc.sync.dma_start(out=w1_sb, in_=w1_d)
            for dchunk in range(4):
                nc.sync.dma_start(
                    out=xq[:, 14 * dchunk : 14 * (dchunk + 1), :],
                    in_=xbf_d[0][:, 14 * dchunk : 14 * (dchunk + 1), :],
                )
            for cchunk in range(BANKS):
                nc.vector.tensor_copy(
                    out=x_pad[0][:, 1 + RPB * cchunk : 1 + RPB * (cchunk + 1), 1 : W + 1],
                    in_=xq[:, RPB * cchunk : RPB * (cchunk + 1), :],
                )

            # Hoist the ACT table load off the critical path: the first
            # ACTIVATE in the Scalar stream triggers it.
            nc.scalar.activation(out=dummy, in_=dummy, func=relu)

            # Warm up the PE's HAM clock gate while image 0's DMA is in
            # flight: a few throwaway matmuls keep the PE busy so the real
            # ones start at (or near) 2.4 GHz without delaying them.
            wps = psum_pool.tile([C, 448], F32, name="ps")
            for wi in range(7):
                nc.tensor.matmul(
                    wps,
                    lhsT=warm[:, 0:C],
                    rhs=warm[:, :],
                    start=(wi == 0),
                    stop=(wi == 6),
                )

            nc.sync.dma_start(out=b1_sb, in_=b1_d)
            nc.sync.dma_start(out=b2_sb, in_=b2_d)
            nc.sync.dma_start(out=w2_sb, in_=w2_d)

            # Zero the remaining buffers' borders (still GpSimd — nothing
            # else runs there; the image-0 scatter copies moved to VectorE
            # so these can't delay them). Interiors are fully rewritten per
            # image.
            for buf in x_pad[1:] + mid_pad[1:]:
                zero_borders(buf)

            for i in range(IMGS):
                d = i % DEPTH
                xp, mp, ob = x_pad[d], mid_pad[d], out_sb[d]
                if i > 0:
                    nc.sync.dma_start(
                        out=xp[:, 1 : H + 1, 1 : W + 1], in_=xbf_d[i]
                    )

                # Banks are processed in pairs sharing each tap's weights:
                # consecutive matmuls with the same stationary operand let
                # the weight load be reused/overlapped.
                pairs = [
                    tuple(b for b in (p, p + 1) if b < BANKS)
                    for p in range(0, BANKS, 2)
                ]

                # conv1 + bn1 + relu -> mid (bf16, padded)
                for pair in pairs:
                    pts = [psum_pool.tile([C, RPB, W], F32, name="ps") for _ in pair]
                    for kk in range(KK):
                        ky, kx = divmod(kk, 3)
                        for ps, b in zip(pts, pair):
                            nc.tensor.matmul(
                                ps,
                                lhsT=w1_sb[:, kk, :],
                                rhs=xp[
                                    :, RPB * b + ky : RPB * b + ky + RPB, kx : kx + W
                                ],
                                start=(kk == 0),
                                stop=(kk == KK - 1),
                            )
                    for ps, b in zip(pts, pair):
                        nc.scalar.activation(
                            out=mp[:, 1 + RPB * b : 1 + RPB * (b + 1), 1 : W + 1],
                            in_=ps,
                            func=relu,
                            bias=b1_sb[:, 0:1],
                        )

                # conv2 + bn2 + residual + relu -> out
                for pair in pairs:
                    pts2 = [psum_pool.tile([C, RPB, W], F32, name="ps") for _ in pair]
                    for kk in range(KK):
                        ky, kx = divmod(kk, 3)
                        for ps2, b in zip(pts2, pair):
                            nc.tensor.matmul(
                                ps2,
                                lhsT=w2_sb[:, kk, :],
                                rhs=mp[
                                    :, RPB * b + ky : RPB * b + ky + RPB, kx : kx + W
                                ],
                                start=(kk == 0),
                                stop=(kk == KK - 1),
                            )
                    for ps2, b in zip(pts2, pair):
                        last_bank = i == IMGS - 1 and b == BANKS - 1
                        # The final bank's epilogue + store is the serial
                        # tail after the last matmul; split it into 4-row
                        # halves so the first half's DMA overlaps the rest.
                        halves = (
                            [(0, RPB // 2), (RPB // 2, RPB)]
                            if last_bank
                            else [(0, RPB)]
                        )
                        for r0, r1 in halves:
                            rows = ob[:, RPB * b + r0 : RPB * b + r1, :]
                            nc.vector.scalar_tensor_tensor(
                                out=rows,
                                in0=ps2[:, r0:r1, :],
                                scalar=b2_sb[:, 0:1],
                                in1=xp[
                                    :, 1 + RPB * b + r0 : 1 + RPB * b + r1, 1 : W + 1
                                ],
                                op0=add,
                                op1=add,
                            )
                            nc.vector.tensor_scalar_max(rows, rows, 0.0)
                            if i == IMGS - 1:
                                nc.scalar.dma_start(
                                    out=out_d[i][:, RPB * b + r0 : RPB * b + r1, :],
                                    in_=rows,
                                )

                if i < IMGS - 1:
                    nc.scalar.dma_start(out=out_d[i], in_=ob)

    nc.compile()
    return nc


def _install_neff_cache():
    """Content-addressed on-disk cache for walrus NEFF compiles.

    The BIR JSON for this module is byte-identical across processes, so a
    fresh process can reuse the NEFF compiled by an earlier one instead of
    paying the multi-minute walrus compile again.
    """
    import hashlib
    import shutil

    from concourse import bass2jax, bass_utils as bu

    if getattr(bu, "_neff_cache_installed", False):
        return
    orig = bu.compile_bir_kernel
    cache_dir = "/var/tmp/bass_neff_cache"

    def cached(bir_json, tmpdir, neff_name="file.neff"):
        data = bir_json if isinstance(bir_json, bytes) else bir_json.encode()
        key = hashlib.sha256(data).hexdigest()
        cpath = os.path.join(cache_dir, key + ".neff")
        try:
            if os.path.exists(cpath):
                dst = os.path.join(tmpdir, neff_name)
                shutil.copy(cpath, dst)
                return dst
        except OSError:
            pass
        neff_path = orig(bir_json, tmpdir, neff_name)
        try:
            os.makedirs(cache_dir, exist_ok=True)
            tmp = cpath + f".tmp{os.getpid()}"
            shutil.copy(neff_path, tmp)
            os.replace(tmp, cpath)
        except OSError:
            pass
        return neff_path

    bu.compile_bir_kernel = cached
    bass2jax.compile_bir_kernel = cached
    bu._neff_cache_installed = True


@functools.lru_cache(maxsize=1)
def _get_module():
    _install_neff_cache()
    return _build_module()


def _prep_in_maps(inputs):
    f32 = np.float32
    x = np.asarray(inputs["x"], f32)
    w1 = np.asarray(inputs["w1"], f32)
    w2 = np.asarray(inputs["w2"], f32)
    gamma1 = np.asarray(inputs["gamma1"], f32)
    beta1 = np.asarray(inputs["beta1"], f32)
    mean1 = np.asarray(inputs["mean1"], f32)
    var1 = np.asarray(inputs["var1"], f32)
    gamma2 = np.asarray(inputs["gamma2"], f32)
    beta2 = np.asarray(inputs["beta2"], f32)
    mean2 = np.asarray(inputs["mean2"], f32)
    var2 = np.asarray(inputs["var2"], f32)

    a1 = gamma1 / np.sqrt(var1 + EPS)
    a2 = gamma2 / np.sqrt(var2 + EPS)
    # Fold BN scale into weights; transpose to [c_in, tap, c_out] for lhsT.
    w1t = np.ascontiguousarray(
        np.transpose(w1 * a1[:, None, None, None], (1, 2, 3, 0)).reshape(C, KK, C)
    ).astype(ml_dtypes.bfloat16)
    w2t = np.ascontiguousarray(
        np.transpose(w2 * a2[:, None, None, None], (1, 2, 3, 0)).reshape(C, KK, C)
    ).astype(ml_dtypes.bfloat16)
    b1 = np.ascontiguousarray((beta1 - mean1 * a1).reshape(C, 1).astype(f32))
    b2 = np.ascontiguousarray((beta2 - mean2 * a2).reshape(C, 1).astype(f32))

    xbf = np.ascontiguousarray(x).astype(ml_dtypes.bfloat16)
    return [
        {
            "xbf": xbf[IMGS * i : IMGS * (i + 1)],
            "w1t": w1t,
            "w2t": w2t,
            "b1": b1,
            "b2": b2,
        }
        for i in range(N_CORES)
    ]


def _run(inputs, trace=False):
    nc = _get_module()
    in_maps = _prep_in_maps(inputs)
    res = bass_utils.run_bass_kernel_spmd(
        nc, in_maps, core_ids=list(range(N_CORES)), trace=trace
    )
    out = np.concatenate([r["out"] for r in res.results], axis=0)
    return out.astype(np.float32), res


def kernel(**inputs):
    out, _ = _run(inputs, trace=False)
    return out

